# revision 21
# baseline (speedup 1.0000x reference)
"""Trainium2 Bass kernel for nn_Attention_52536039965434 (v4).

Reference computation (B=2, SQ=SK=2048, H=1024, NH=16, HD=64):
    qkv = x @ c_attn_w + b ; per-head attention with multiplicative mask
    (post-score, pre-softmax); attn @ c_proj_w + b; gelu(cat(x, attn) @ mlp_w + b)

Key algorithmic points (v4):
  * k-COMPACTION: the mask is multiplicative 0/1 applied to scores PRE-softmax,
    so masked k contribute exp(0)=1 * V_k.  We compact the k axis to the
    ~1000 unmasked positions (SP=1152 padded slots) and add ONE phantom slot
    with x_phantom = sum(masked x), aug-row = N_masked, and denominator-column
    value N_masked.  Pad slots have V-row and denominator-column 0, so they
    contribute nothing.  Exact (just a reordering of the softmax sums).
  * Sharding: core c -> (b=c//4, g=c%4). Data parallel over batch; attention
    tensor-parallel over 4 head-groups.  Each core processes q-blocks in the
    VIRTUAL order (g+1, g+2, g+3, g) (host permutes xatt columns), publishes
    its attnT piece for steps 0-2 via a per-step 4-way AllGather (overlapped
    under later attention steps), and keeps the step-3 (own-quarter) piece
    local.  The per-core receive offsets (which rank's piece to use from each
    gathered buffer) are runtime data: a dma_gather with per-core int16
    indices + per-core permuted c_proj weight rows.  Then c_proj contracts
    the FULL 1024 attn dims for the own q-quarter, and mlp2 follows locally.
    No big end-of-kernel collective (v3's ReduceScatter of z cost ~100us).
  * Softmax denominator rides as V's 65th column (values from vcol input);
    normalize uses a PE ones-broadcast + DVE multiply so the gpsimd queue
    (which hosts the collectives) is never on the attention critical path.
"""

import os

import numpy as np
import ml_dtypes

import concourse.bacc as bacc
import concourse.mybir as mybir
import concourse.tile as tile
from concourse import bass_utils

# ---- problem dims (hardcoded per contest contract) ----
B = 2
S = 2048          # SQ == SK
H = 1024
NH = 16
HD = 64
NCORES = 8
TP = 4            # cores per batch (head groups / q-quarters)
HPC = NH // TP    # heads per core = 4
DH = HPC * HD     # per-core head width = 256
QB = 512          # q-block (matmul moving free dim)
P = 128
# Compacted k-slot count (unmasked ~1024 + phantom + pad).  1152 covers the
# fixed-seed inputs (1061/1019 unmasked on CPU); kernel() falls back to the
# 1280 build if a mask with more unmasked slots ever shows up.
SP_OPTIONS = (1152, 1280)

F32 = mybir.dt.float32
F32R = mybir.dt.float32r
BF16 = mybir.dt.bfloat16
I16 = mybir.dt.int16
AF = mybir.ActivationFunctionType
ALU = mybir.AluOpType
NPBF16 = ml_dtypes.bfloat16


def _build_nc(reps=1, sp=1024):
    """Build + compile the single SPMD Bass program (same NEFF on all 8 cores)."""
    skip_ag = os.environ.get("KERNEL_AG", "cc") != "cc"
    SP = sp
    NKT = SP // P                       # k tiles
    KB = next(kb for kb in (512, 384, 320, 256, 128) if SP % kb == 0)
    NKB = SP // KB
    s = S
    nq = s // QB          # q blocks = 4
    nf = H // P           # feature tiles of H = 8

    nc = bacc.Bacc(
        "TRN2", target_bir_lowering=False, debug=False, num_devices=NCORES
    )

    # ---- kernel I/O (per-core contents supplied via in_maps) ----
    xatt = nc.dram_tensor("xatt", [H, s], BF16, kind="ExternalInput").ap()
    xatd = nc.dram_tensor("xatd", [H + 1, SP], BF16, kind="ExternalInput").ap()
    xmlp_d = nc.dram_tensor("xmlp", [H, QB], BF16, kind="ExternalInput").ap()
    wq_d = nc.dram_tensor("wq", [H, DH], BF16, kind="ExternalInput").ap()
    wk_d = nc.dram_tensor("wk", [H, DH], BF16, kind="ExternalInput").ap()
    wv_d = nc.dram_tensor("wv", [H + 1, DH], BF16, kind="ExternalInput").ap()
    qkb_d = nc.dram_tensor("qkbias", [P, 4], F32, kind="ExternalInput").ap()
    beff_d = nc.dram_tensor("beff", [P, nf], F32, kind="ExternalInput").ap()
    kmask_d = nc.dram_tensor("kmask", [1, SP], F32, kind="ExternalInput").ap()
    vcol_d = nc.dram_tensor("vcol", [P, NKT], BF16, kind="ExternalInput").ap()
    gidx_d = nc.dram_tensor("gidx", [P, 3 * 8], I16, kind="ExternalInput").ap()
    ones_d = nc.dram_tensor("ones1", [1, 64], F32R, kind="ExternalInput").ap()
    cpw_d = nc.dram_tensor("cprojw", [H, H], BF16, kind="ExternalInput").ap()
    mw1_d = nc.dram_tensor("mlpw1", [H, H], BF16, kind="ExternalInput").ap()
    mw2_d = nc.dram_tensor("mlpw2", [H, H], BF16, kind="ExternalInput").ap()
    outQ = nc.dram_tensor("outQ", [H, QB], BF16, kind="ExternalOutput").ap()

    rg = [[0, 1, 2, 3], [4, 5, 6, 7]]

    with tile.TileContext(nc) as tc:
      for rep in range(reps):
        with (
            tc.tile_pool(name=f"dram{rep}", bufs=1, space="DRAM") as dram,
            tc.tile_pool(name=f"w{rep}", bufs=1) as wpool,
            tc.tile_pool(name=f"per{rep}", bufs=1) as per,
            tc.tile_pool(name=f"xstream{rep}", bufs=4) as xstream,
            tc.tile_pool(name=f"qt{rep}", bufs=2) as qtpool,
            tc.tile_pool(name=f"at{rep}", bufs=2) as atpool,
            tc.tile_pool(name=f"e{rep}", bufs=4) as epool,
            tc.tile_pool(name=f"go{rep}", bufs=1) as gopool,
            tc.tile_pool(name=f"small{rep}", bufs=2) as small,
            tc.tile_pool(name=f"psg{rep}", bufs=2, space="PSUM") as psg,
            tc.tile_pool(name=f"pss{rep}", bufs=2, space="PSUM") as pss,
            tc.tile_pool(name=f"psv{rep}", bufs=2, space="PSUM") as psv,
        ):
            # DRAM staging for the per-step AllGather exchange
            agin = [
                dram.tile([P, 2 * QB], BF16, tag="agin", name=f"agin{rep}_{t}")
                for t in range(3)
            ]
            agout = dram.tile([3 * TP * P, 2 * QB], BF16, tag="agout",
                              name=f"agout{rep}")

            # ---------- weight / bias / mask loads (Pool-queue SWDGE, so the
            # SP HWDGE path is free for the x streams) ----------
            wq_sb = wpool.tile([P, nf * DH], BF16, tag="wq")
            wk_sb = wpool.tile([P, nf * DH], BF16, tag="wk")
            wv_sb = wpool.tile([P, nf * DH], BF16, tag="wv")
            wvb_sb = wpool.tile([1, DH], BF16, tag="wvb")
            qkb_sb = wpool.tile([P, 4], F32, tag="qkb")
            beff_sb = wpool.tile([P, nf], F32, tag="beff")
            mask_sb = wpool.tile([P, SP], F32, tag="mask")
            mask_row = wpool.tile([1, SP], F32, tag="maskrow")
            gidx_sb = wpool.tile([P, 3 * 8], I16, tag="gidx")
            ones_sb = wpool.tile([1, 64], F32R, tag="ones")
            cpw_sb = wpool.tile([P, nf * H], BF16, tag="cpw")
            mw1_sb = wpool.tile([P, nf * H], BF16, tag="mw1")
            mw2_sb = wpool.tile([P, nf * H], BF16, tag="mw2")
            xmlp_sb = wpool.tile([P, nf * QB], BF16, tag="xmlp")
            xatd_sb = wpool.tile([P, nf * SP], BF16, tag="xatd")
            xaug_sb = wpool.tile([1, SP], BF16, tag="xaug")

            nc.gpsimd.dma_start(
                out=wk_sb[:].rearrange("p (t d) -> p t d", d=DH),
                in_=wk_d[:H].rearrange("(t p) d -> p t d", p=P),
            )
            nc.gpsimd.dma_start(out=qkb_sb[:], in_=qkb_d[:])
            nc.gpsimd.dma_start(out=mask_row[:], in_=kmask_d[0:1])
            nc.gpsimd.partition_broadcast(mask_sb[:], mask_row[:], channels=P)
            nc.gpsimd.dma_start(out=gidx_sb[:], in_=gidx_d[:])
            nc.gpsimd.dma_start(out=ones_sb[:], in_=ones_d[:])

            def emit_xatd_dmas():
                # attendee x (compacted), KB-column chunks so K(kb0) can
                # start after ~1/NKB of the transfer; queued on sync AFTER
                # the first xq pair (which gates QT(0)).
                for kb in range(NKB):
                    cs = slice(kb * KB, (kb + 1) * KB)
                    for i in range(2):
                        nc.sync.dma_start(
                            out=xatd_sb[:, i * (nf // 2) * SP : (i + 1) * (nf // 2) * SP]
                            .rearrange("p (t q) -> p t q", q=SP)[:, :, cs],
                            in_=xatd[i * (H // 2) : (i + 1) * (H // 2)]
                            .rearrange("(t p) q -> p t q", p=P)[:, :, cs],
                        )
                nc.gpsimd.dma_start(out=xaug_sb[:], in_=xatd[H : H + 1])
            nc.gpsimd.dma_start(
                out=wv_sb[:].rearrange("p (t d) -> p t d", d=DH),
                in_=wv_d[:H].rearrange("(t p) d -> p t d", p=P),
            )
            nc.gpsimd.dma_start(out=wvb_sb[:], in_=wv_d[H : H + 1])
            nc.gpsimd.dma_start(
                out=wq_sb[:].rearrange("p (t d) -> p t d", d=DH),
                in_=wq_d[:H].rearrange("(t p) d -> p t d", p=P),
            )

            def late_weight_dmas():
                nc.gpsimd.dma_start(
                    out=cpw_sb[:].rearrange("p (d o) -> p d o", o=H),
                    in_=cpw_d[:].rearrange("(d p) o -> p d o", p=P),
                )
                nc.gpsimd.dma_start(
                    out=xmlp_sb[:].rearrange("p (t q) -> p t q", q=QB),
                    in_=xmlp_d[:].rearrange("(t p) q -> p t q", p=P),
                )
                for w_d, w_sb in ((mw1_d, mw1_sb), (mw2_d, mw2_sb)):
                    nc.gpsimd.dma_start(
                        out=w_sb[:].rearrange("p (t o) -> p t o", o=H),
                        in_=w_d[:].rearrange("(t p) o -> p t o", p=P),
                    )
                nc.gpsimd.dma_start(out=beff_sb[:], in_=beff_d[:])

            # ---------- persistent activations ----------
            KT_sb = per.tile([P, 2 * SP], BF16, tag="kt")    # pair p at cols p*SP
            V_sb = per.tile([P, NKT * 260], BF16, tag="v")   # per kt: 4 heads x 65
            out1_sb = per.tile([P, nf * QB], BF16, tag="out1")  # ot*QB + q
            attrcv = per.tile([P, 3 * 2 * QB], BF16, tag="attrcv")
            zpart_sb = per.tile([P, nf * QB], BF16, tag="zpart")
            z_sb = per.tile([P, nf * QB], BF16, tag="zs")

            # denominator column of augmented V (1 / N_masked / 0 per slot)
            for h in range(HPC):
                nc.gpsimd.dma_start(
                    out=V_sb[:].rearrange("p (t x) -> p t x", x=260)
                    [:, :, h * 65 + 64 : h * 65 + 65],
                    in_=vcol_d[:].rearrange("p (t o) -> p t o", o=1),
                )

            # ---------- emission units ----------
            def kv_k_unit(kb, p):
                def emit():
                    cs = slice(kb * KB, (kb + 1) * KB)
                    ps = psg.tile([P, QB], F32, tag="g", name=f"kps{kb}_{p}")
                    for t in range(nf):
                        nc.tensor.matmul(
                            ps[:, :KB],
                            lhsT=w_slice(wk_sb, t, p),
                            rhs=xatd_sb[:, t * SP + kb * KB : t * SP + (kb + 1) * KB],
                            start=(t == 0),
                            stop=(t == nf - 1),
                        )
                    # evacuation: (k + bias) * mask in one DVE op
                    nc.vector.scalar_tensor_tensor(
                        KT_sb[:, p * SP + kb * KB : p * SP + (kb + 1) * KB],
                        ps[:, :KB],
                        qkb_sb[:, 2 + p : 3 + p],
                        mask_sb[:, cs],
                        ALU.add,
                        ALU.mult,
                    )
                return emit

            def kv_v_unit(kt):
                def emit():
                    pv_ps = psg.tile([P, QB], F32, tag="g", name=f"vps{kt}")
                    for t in range(nf):
                        nc.tensor.matmul(
                            pv_ps[:, :DH],
                            lhsT=xatd_sb[:, t * SP + kt * P : t * SP + (kt + 1) * P],
                            rhs=wv_sb[:, t * DH : (t + 1) * DH],
                            start=(t == 0),
                            stop=False,
                        )
                    nc.tensor.matmul(
                        pv_ps[:, :DH],
                        lhsT=xaug_sb[0:1, kt * P : (kt + 1) * P],
                        rhs=wvb_sb[:],
                        start=False,
                        stop=True,
                    )
                    nc.vector.tensor_copy(
                        V_sb[:, kt * 260 : (kt + 1) * 260]
                        .rearrange("p (h c) -> p h c", c=65)[:, :, 0:64],
                        pv_ps[:, :DH].rearrange("p (h c) -> p h c", c=HD),
                    )
                return emit

            xq_chs = {}

            def qt_units(qb):
                """QT output-half units for qb; even qb's first unit DMAs the
                1024-col x pair (qb, qb+1)."""
                qo = qb % 2
                if qb % 2 == 0:
                    xq_chs[qb // 2] = [
                        xstream.tile([P, (nf // 2) * 2 * QB], BF16, tag="xch",
                                     name=f"xq{qb}_{i}")
                        for i in range(2)
                    ]
                x_ch = xq_chs[qb // 2]
                QT_t = qtpool.tile([P, 2 * QB], BF16, tag="qt",
                                   name=f"qt{qb}")
                def emit_p(p, dma=False):
                    def emit():
                        if dma:
                            cs = slice((qb // 2) * 2 * QB,
                                       (qb // 2 + 1) * 2 * QB)
                            for i in range(2):
                                nc.sync.dma_start(
                                    out=x_ch[i][:].rearrange(
                                        "p (t q) -> p t q", q=2 * QB),
                                    in_=xatt[i * (H // 2) : (i + 1) * (H // 2)]
                                    .rearrange("(t p) q -> p t q", p=P)[:, :, cs],
                                )
                        ps = psg.tile([P, QB], F32, tag="g", name=f"qps{qb}_{p}")
                        for t in range(nf):
                            nc.tensor.matmul(
                                ps[:],
                                lhsT=w_slice(wq_sb, t, p),
                                rhs=x_ch[t // 4][
                                    :, (t % 4) * 2 * QB + qo * QB :
                                    (t % 4) * 2 * QB + (qo + 1) * QB],
                                start=(t == 0),
                                stop=(t == nf - 1),
                            )
                        nc.vector.tensor_scalar(
                            QT_t[:, p * QB : (p + 1) * QB],
                            ps[:],
                            qkb_sb[:, p : p + 1],
                            None,
                            ALU.add,
                        )
                    return emit
                return QT_t, [emit_p(0, dma=(qb % 2 == 0)), emit_p(1)]

            def mlp1_unit(ot):
                def emit():
                    ps = psg.tile([P, QB], F32, tag="g", name=f"m1ps{ot}")
                    for t in range(nf):
                        nc.tensor.matmul(
                            ps[:],
                            lhsT=mw1_sb[:, t * H + ot * P : t * H + (ot + 1) * P],
                            rhs=xmlp_sb[:, t * QB : (t + 1) * QB],
                            start=(t == 0),
                            stop=(t == nf - 1),
                        )
                    nc.vector.tensor_copy(
                        out1_sb[:, ot * QB : (ot + 1) * QB], ps[:]
                    )
                return emit

            fillers = []

            def pop_filler():
                if fillers:
                    fillers.pop(0)()

            def attention(tq, QT_t, attnT_t, pre_kt=None):
                for p in range(2):
                    pvs = [
                        psv.tile([65, QB], F32, tag="pv",
                                 name=f"pv{tq}_{p}_{h}")
                        for h in range(2)
                    ]
                    sc_tiles = {}

                    def emit_scores(kt, p=p, tq=tq, QT_t=QT_t,
                                    sc_tiles=sc_tiles):
                        if pre_kt is not None:
                            pre_kt(kt)
                        sc = pss.tile([P, 2 * QB], F32, tag="sc",
                                      name=f"sc{tq}_{p}_{kt}")
                        sc_tiles[kt] = sc
                        for half in range(2):
                            nc.tensor.matmul(
                                sc[:, half * QB : (half + 1) * QB],
                                lhsT=KT_sb[
                                    64 * half : 64 * half + 64,
                                    p * SP + kt * P : p * SP + (kt + 1) * P,
                                ],
                                rhs=QT_t[64 * half : 64 * half + 64,
                                         p * QB : (p + 1) * QB],
                                start=True,
                                stop=True,
                                tile_position=(64 * half, 0),
                            )

                    emit_scores(0)
                    for kt in range(NKT):
                        if kt + 1 < NKT:
                            emit_scores(kt + 1)
                        pop_filler()
                        e = epool.tile([P, 2 * QB], BF16, tag="e")
                        nc.scalar.activation(e[:], sc_tiles.pop(kt)[:], AF.Exp)
                        for half in range(2):
                            h = 2 * p + half
                            nc.tensor.matmul(
                                pvs[half][:],
                                lhsT=V_sb[:, kt * 260 + h * 65 : kt * 260
                                          + (h + 1) * 65],
                                rhs=e[:, half * QB : (half + 1) * QB],
                                start=(kt == 0),
                                stop=(kt == NKT - 1),
                            )
                    # normalize by the denominator (row 64), store attn^T.
                    # rec broadcast via a PE ones-matmul (NOT gpsimd, which
                    # hosts the collectives and may block).
                    for half in range(2):
                        rec = small.tile([1, QB], F32R, tag="rec")
                        with nc.allow_low_precision(
                            reason="fp32r reciprocal for PE broadcast; "
                            "2^-11 rel err on softmax denom is fine"
                        ):
                            nc.vector.reciprocal(rec[:], pvs[half][64:65, :])
                        recb = psg.tile([P, QB], F32, tag="g",
                                        name=f"recb{tq}_{p}_{half}")
                        nc.tensor.matmul(
                            recb[0:64, :],
                            lhsT=ones_sb[:],
                            rhs=rec[:],
                            start=True,
                            stop=True,
                        )
                        recs = small.tile([64, QB], F32, tag="recs")
                        nc.vector.tensor_copy(recs[:], recb[0:64, :])
                        nc.vector.tensor_tensor(
                            attnT_t[64 * half : 64 * half + 64,
                                    p * QB : (p + 1) * QB],
                            pvs[half][0:64, :],
                            recs[:],
                            ALU.mult,
                        )

            # ---------- PE warm-up: release the HAM clock-gate (~3.4us of
            # activity) while the first x DMAs are still in flight ----------
            junk = small.tile([P, QB], BF16, tag="junk")
            nc.vector.memset(junk[:], 0.5)
            warm_ps = psg.tile([P, QB], F32, tag="g", name="warm")
            for i in range(10):
                nc.tensor.matmul(
                    warm_ps[:], lhsT=junk[:, :P], rhs=junk[:],
                    start=(i == 0), stop=(i == 9),
                )

            # ---------- emission schedule ----------
            QT_ts = {}
            QT_ts[0], qt0_units = qt_units(0)
            qt0_units[0]()          # emits the xq pair-0 DMA first
            emit_xatd_dmas()
            qt0_units[1]()

            kunits = {kb: [kv_k_unit(kb, 0), kv_k_unit(kb, 1)]
                      for kb in range(NKB)}
            vunits = {kt: kv_v_unit(kt) for kt in range(NKT)}

            def ensure_kv(kt):
                """Emit K-block and V units needed by scores/attnV at kt."""
                for b in range(min(kt // 3 + 1, NKB)):
                    for u in kunits.pop(b, ()):
                        u()
                for k2 in range(kt + 1):
                    u = vunits.pop(k2, None)
                    if u is not None:
                        u()

            def cproj_rhs(d):
                sl, pp = d // 2, d % 2
                return (
                    attrcv[:, sl * 2 * QB + pp * QB : sl * 2 * QB + (pp + 1) * QB]
                    if sl < 3
                    else attnT_ts[3][:, pp * QB : (pp + 1) * QB]
                )

            def cproj_a_unit(ot):
                """Contraction slots 0-1 (pieces gathered after steps 0/1);
                runs as PE filler during attention step 3."""
                def emit():
                    ps = psg.tile([P, QB], F32, tag="g", name=f"cpa{ot}")
                    for d in range(4):
                        nc.tensor.matmul(
                            ps[:],
                            lhsT=cpw_sb[:, d * H + ot * P : d * H + (ot + 1) * P],
                            rhs=cproj_rhs(d),
                            start=(d == 0),
                            stop=(d == 3),
                        )
                    nc.vector.tensor_copy(
                        zpart_sb[:, ot * QB : (ot + 1) * QB], ps[:]
                    )
                return emit

            mlp1_per_qb = {1: [0, 1, 2], 2: [3, 4, 5], 3: [6, 7]}
            attnT_ts = {}
            for tq in range(nq):
                if tq == 1:
                    late_weight_dmas()
                if tq + 1 < nq:
                    QT_ts[tq + 1], units = qt_units(tq + 1)
                    fillers.extend(units)
                for ot in mlp1_per_qb.get(tq, []):
                    fillers.append(mlp1_unit(ot))
                if tq == 3:
                    fillers.extend(cproj_a_unit(ot) for ot in range(nf))
                attnT_ts[tq] = atpool.tile([P, 2 * QB], BF16, tag="at",
                                           name=f"at{tq}")
                attention(tq, QT_ts[tq], attnT_ts[tq],
                          pre_kt=ensure_kv if tq == 0 else None)
                if tq < 3:
                    # publish the piece + AllGather it (overlaps later steps),
                    # then pull this core's rank-row via dma_gather
                    nc.sync.dma_start(out=agin[tq][:], in_=attnT_ts[tq][:])
                    if skip_ag:
                        nc.gpsimd.dma_start(
                            out=attrcv[:, tq * 2 * QB : (tq + 1) * 2 * QB],
                            in_=agin[tq][:],
                        )
                    else:
                        nc.gpsimd.collective_compute(
                            "AllGather", ALU.bypass, replica_groups=rg,
                            ins=[agin[tq][:].opt()],
                            outs=[agout[tq * TP * P : (tq + 1) * TP * P, :].opt()],
                        )
                        nc.gpsimd.dma_gather(
                            attrcv[:, tq * 2 * QB : (tq + 1) * 2 * QB]
                            .rearrange("p (o c) -> p o c", o=1),
                            agout[tq * TP * P : (tq + 1) * TP * P, :],
                            gidx_sb[:, tq * 8 : (tq + 1) * 8],
                            P,
                            P,
                            2 * QB,
                        )
            while fillers:
                pop_filler()

            # ---------- c_proj tail: slots 2-3, add the filler partial ------
            for ot in range(nf):
                ps = psg.tile([P, QB], F32, tag="g", name=f"cpb{ot}")
                for d in range(4, nf):
                    nc.tensor.matmul(
                        ps[:],
                        lhsT=cpw_sb[:, d * H + ot * P : d * H + (ot + 1) * P],
                        rhs=cproj_rhs(d),
                        start=(d == 4),
                        stop=(d == nf - 1),
                    )
                nc.vector.tensor_tensor(
                    z_sb[:, ot * QB : (ot + 1) * QB],
                    ps[:],
                    zpart_sb[:, ot * QB : (ot + 1) * QB],
                    ALU.add,
                )

            # ---------- mlp2 (full 1024 outs, own q-quarter) ----------
            gout = gopool.tile([P, 2 * QB], BF16, tag="gout")
            for ot in range(nf):
                ps = psg.tile([P, QB], F32, tag="g", name=f"m2ps{ot}")
                for t in range(nf):
                    nc.tensor.matmul(
                        ps[:],
                        lhsT=mw2_sb[:, t * H + ot * P : t * H + (ot + 1) * P],
                        rhs=z_sb[:, t * QB : (t + 1) * QB],
                        start=(t == 0),
                        stop=(t == nf - 1),
                    )
                o1 = out1_sb[:, ot * QB : (ot + 1) * QB]
                nc.vector.tensor_add(o1, ps[:], o1)

                # gelu (+ folded mlp/cproj bias), batched output DMA
                g = gout[:, (ot % 2) * QB : (ot % 2 + 1) * QB]
                nc.scalar.activation(
                    g, o1, AF.Gelu_apprx_tanh, bias=beff_sb[:, ot : ot + 1]
                )
                if ot % 2 == 1:
                    nc.sync.dma_start(
                        out=outQ[(ot - 1) * P : (ot + 1) * P, :]
                        .rearrange("(t p) q -> p t q", p=P),
                        in_=gout[:].rearrange("p (t q) -> p t q", q=QB),
                    )

    nc.compile()
    return nc


def w_slice(w_sb, t, p):
    """lhsT [128, 128] slice: f-tile t, output half p, of a [128, nt*256] layout."""
    return w_sb[:, t * DH + p * P : t * DH + (p + 1) * P]


_NC_CACHE = {}
LAST_RESULTS = None


def _get_nc_reps(reps, sp=SP_OPTIONS[0]):
    key = ("reps", reps, sp)
    if key not in _NC_CACHE:
        _NC_CACHE[key] = _build_nc(reps=reps, sp=sp)
    return _NC_CACHE[key]


def _pick_sp(inputs):
    mask = np.asarray(inputs["attendee_mask"]).astype(bool)
    need = int(mask.sum(1).max()) + 1  # unmasked slots + phantom
    for sp in SP_OPTIONS:
        if need <= sp:
            return sp
    raise AssertionError(f"mask needs {need} slots > {SP_OPTIONS[-1]}")


def kernel(**inputs):
    global LAST_RESULTS
    sp = _pick_sp(inputs)
    nc = _get_nc_reps(1, sp)
    in_maps = make_in_maps(inputs, sp)

    trace = bool(int(os.environ.get("KERNEL_TRACE", "0")))
    res = bass_utils.run_bass_kernel_spmd(
        nc, in_maps, core_ids=list(range(NCORES)), trace=trace
    )
    LAST_RESULTS = res

    out = np.empty((B, S, H), np.float32)
    for c in range(NCORES):
        b, g = c // TP, c % TP
        out[b, g * QB : (g + 1) * QB, :] = (
            res.results[c]["outQ"].astype(np.float32).T)
    return out


def make_in_maps(inputs, sp=SP_OPTIONS[0]):
    SP = sp
    NKT = SP // P
    xq = np.ascontiguousarray(np.asarray(inputs["attender_seq"], np.float32))
    xk = np.ascontiguousarray(np.asarray(inputs["attendee_seq"], np.float32))
    mask = np.asarray(inputs["attendee_mask"]).astype(bool)
    caw = np.asarray(inputs["c_attn_w"], np.float32)
    cab = np.asarray(inputs["c_attn_b"], np.float32)
    cpw = np.ascontiguousarray(np.asarray(inputs["c_proj_w"], np.float32))
    cpb = np.asarray(inputs["c_proj_b"], np.float32)
    mw = np.ascontiguousarray(np.asarray(inputs["mlp_w"], np.float32))
    mb = np.asarray(inputs["mlp_b"], np.float32)

    mw1_bf = mw[:H].astype(NPBF16)
    mw2_bf = mw[H:].astype(NPBF16)
    # gelu bias: mlp_b + c_proj_b @ mlp_w2  (folded host-side)
    beff = (
        mb.astype(np.float64) + cpb.astype(np.float64) @ mw[H:].astype(np.float64)
    ).astype(np.float32)
    beff_t = np.ascontiguousarray(beff.reshape(H // P, P).T)
    cpw_bf = cpw.astype(NPBF16)

    # per-batch compaction of the attendee axis
    batch_atd = []
    for b in range(B):
        idx = np.flatnonzero(mask[b])
        n_u = len(idx)
        n_masked = S - n_u
        assert n_u + 1 <= SP, (n_u, SP)
        xatd = np.zeros((H + 1, SP), np.float32)
        xatd[:H, :n_u] = xk[b, idx].T
        xatd[:H, n_u] = xk[b].sum(0) - xk[b, idx].sum(0)  # sum of masked x
        xatd[H, :n_u] = 1.0
        xatd[H, n_u] = float(n_masked)
        kmask = np.zeros((1, SP), np.float32)
        kmask[0, :n_u] = 1.0
        vcol = np.zeros(SP, np.float32)
        vcol[:n_u] = 1.0
        vcol[n_u] = float(n_masked)
        vcol_t = np.ascontiguousarray(vcol.reshape(NKT, P).T.astype(NPBF16))
        batch_atd.append((xatd.astype(NPBF16), kmask, vcol_t))

    in_maps = []
    for c in range(NCORES):
        b, g = c // TP, c % TP
        gs = slice(g * DH, (g + 1) * DH)
        xatd_bf, kmask, vcol_t = batch_atd[b]
        wv = np.concatenate(
            [caw[:, 2 * H + g * DH : 2 * H + (g + 1) * DH],
             cab[None, 2 * H + g * DH : 2 * H + (g + 1) * DH]], 0)
        # [128, 4]: q bias (2 output halves), k bias (2 output halves)
        qkb = np.stack(
            [cab[gs][: P], cab[gs][P:],
             cab[H + g * DH : H + (g + 1) * DH][: P],
             cab[H + g * DH : H + (g + 1) * DH][P:]], 1)

        # virtual q-block order: own quarter LAST
        vorder = [(g + 1 + t) % TP for t in range(TP)]
        xatt_v = np.concatenate(
            [xq[b, vq * QB : (vq + 1) * QB, :] for vq in vorder], 0).T

        # receive plan: step t's gathered buffer -> rank r=(g-1-t)%4's piece;
        # c_proj contraction slot s uses cpw rows of that rank's dims
        # (slot 3 = own piece, local).
        rts = [(g - 1 - t) % TP for t in range(3)]
        cpw_v = np.concatenate(
            [cpw_bf[r * DH : (r + 1) * DH] for r in rts + [g]], 0)
        gidx = np.zeros((P, 24), np.int16)
        for t in range(3):
            vals = rts[t] * P + np.arange(P, dtype=np.int16)  # rows in buffer t
            for i in range(P):
                gidx[i % 16, t * 8 + i // 16] = vals[i]
        gidx[16:, :] = np.tile(gidx[:16, :], (7, 1))

        in_maps.append({
            "xatt": np.ascontiguousarray(xatt_v.astype(NPBF16)),
            "xatd": np.ascontiguousarray(xatd_bf),
            "xmlp": np.ascontiguousarray(
                xq[b, g * QB : (g + 1) * QB, :].T.astype(NPBF16)),
            "wq": np.ascontiguousarray(caw[:, gs].astype(NPBF16)),
            "wk": np.ascontiguousarray(
                caw[:, H + g * DH : H + (g + 1) * DH].astype(NPBF16)),
            "wv": np.ascontiguousarray(wv.astype(NPBF16)),
            "qkbias": np.ascontiguousarray(qkb),
            "beff": beff_t,
            "kmask": kmask,
            "vcol": vcol_t,
            "gidx": gidx,
            "ones1": np.ones((1, 64), np.float32),
            "cprojw": np.ascontiguousarray(cpw_v),
            "mlpw1": mw1_bf,
            "mlpw2": mw2_bf,
        })
    return in_maps


# revision 22
# speedup vs baseline: 1.3699x; 1.3699x over previous
"""Trainium2 Bass kernel for nn_Attention_52536039965434 (v4).

Reference computation (B=2, SQ=SK=2048, H=1024, NH=16, HD=64):
    qkv = x @ c_attn_w + b ; per-head attention with multiplicative mask
    (post-score, pre-softmax); attn @ c_proj_w + b; gelu(cat(x, attn) @ mlp_w + b)

Key algorithmic points (v4):
  * k-COMPACTION: the mask is multiplicative 0/1 applied to scores PRE-softmax,
    so masked k contribute exp(0)=1 * V_k.  We compact the k axis to the
    ~1000 unmasked positions (SP=1152 padded slots) and add ONE phantom slot
    with x_phantom = sum(masked x), aug-row = N_masked, and denominator-column
    value N_masked.  Pad slots have V-row and denominator-column 0, so they
    contribute nothing.  Exact (just a reordering of the softmax sums).
  * Sharding: core c -> (b=c//4, g=c%4). Data parallel over batch; attention
    tensor-parallel over 4 head-groups.  Each core processes q-blocks in the
    VIRTUAL order (g+1, g+2, g+3, g) (host permutes xatt columns), publishes
    its attnT piece for steps 0-2 via a per-step 4-way AllGather (overlapped
    under later attention steps), and keeps the step-3 (own-quarter) piece
    local.  The per-core receive offsets (which rank's piece to use from each
    gathered buffer) are runtime data: a dma_gather with per-core int16
    indices + per-core permuted c_proj weight rows.  Then c_proj contracts
    the FULL 1024 attn dims for the own q-quarter, and mlp2 follows locally.
    No big end-of-kernel collective (v3's ReduceScatter of z cost ~100us).
  * Softmax denominator rides as V's 65th column (values from vcol input);
    normalize uses a PE ones-broadcast + DVE multiply so the gpsimd queue
    (which hosts the collectives) is never on the attention critical path.
"""

import os

import numpy as np
import ml_dtypes

import concourse.bacc as bacc
import concourse.mybir as mybir
import concourse.tile as tile
from concourse import bass_utils

# ---- problem dims (hardcoded per contest contract) ----
B = 2
S = 2048          # SQ == SK
H = 1024
NH = 16
HD = 64
NCORES = 8
TP = 4            # cores per batch (head groups / q-quarters)
HPC = NH // TP    # heads per core = 4
DH = HPC * HD     # per-core head width = 256
QB = 512          # q-block (matmul moving free dim)
P = 128
# Compacted k-slot count (unmasked ~1024 + phantom + pad).  1152 covers the
# fixed-seed inputs (1061/1019 unmasked on CPU); kernel() falls back to the
# 1280 build if a mask with more unmasked slots ever shows up.
SP_OPTIONS = (1152, 1280)

F32 = mybir.dt.float32
F32R = mybir.dt.float32r
BF16 = mybir.dt.bfloat16
I16 = mybir.dt.int16
AF = mybir.ActivationFunctionType
ALU = mybir.AluOpType
NPBF16 = ml_dtypes.bfloat16


def _build_nc(reps=1, sp=1024):
    """Build + compile the single SPMD Bass program (same NEFF on all 8 cores)."""
    skip_ag = os.environ.get("KERNEL_AG", "cc") != "cc"
    SP = sp
    NKT = SP // P                       # k tiles
    KB = next(kb for kb in (512, 384, 320, 256, 128) if SP % kb == 0)
    NKB = SP // KB
    s = S
    nq = s // QB          # q blocks = 4
    nf = H // P           # feature tiles of H = 8

    nc = bacc.Bacc(
        "TRN2", target_bir_lowering=False, debug=False, num_devices=NCORES
    )

    # ---- kernel I/O (per-core contents supplied via in_maps) ----
    xatt = nc.dram_tensor("xatt", [H, s], BF16, kind="ExternalInput").ap()
    xatd = nc.dram_tensor("xatd", [H + 1, SP], BF16, kind="ExternalInput").ap()
    xmlp_d = nc.dram_tensor("xmlp", [H, QB], BF16, kind="ExternalInput").ap()
    wq_d = nc.dram_tensor("wq", [H, DH], BF16, kind="ExternalInput").ap()
    wk_d = nc.dram_tensor("wk", [H, DH], BF16, kind="ExternalInput").ap()
    wv_d = nc.dram_tensor("wv", [H + 1, DH], BF16, kind="ExternalInput").ap()
    qkb_d = nc.dram_tensor("qkbias", [P, 4], F32, kind="ExternalInput").ap()
    beff_d = nc.dram_tensor("beff", [P, nf], F32, kind="ExternalInput").ap()
    kmask_d = nc.dram_tensor("kmask", [1, SP], F32, kind="ExternalInput").ap()
    vcol_d = nc.dram_tensor("vcol", [P, NKT], BF16, kind="ExternalInput").ap()
    gidx_d = nc.dram_tensor("gidx", [P, 3 * 8], I16, kind="ExternalInput").ap()
    ones_d = nc.dram_tensor("ones1", [1, 64], F32R, kind="ExternalInput").ap()
    cpw_d = nc.dram_tensor("cprojw", [H, H], BF16, kind="ExternalInput").ap()
    mw1_d = nc.dram_tensor("mlpw1", [H, H], BF16, kind="ExternalInput").ap()
    mw2_d = nc.dram_tensor("mlpw2", [H, H], BF16, kind="ExternalInput").ap()
    outQ = nc.dram_tensor("outQ", [H, QB], BF16, kind="ExternalOutput").ap()

    rg = [[0, 1, 2, 3], [4, 5, 6, 7]]

    with tile.TileContext(nc) as tc:
      for rep in range(reps):
        with (
            tc.tile_pool(name=f"dram{rep}", bufs=1, space="DRAM") as dram,
            tc.tile_pool(name=f"w{rep}", bufs=1) as wpool,
            tc.tile_pool(name=f"per{rep}", bufs=1) as per,
            tc.tile_pool(name=f"xstream{rep}", bufs=4) as xstream,
            tc.tile_pool(name=f"qt{rep}", bufs=2) as qtpool,
            tc.tile_pool(name=f"at{rep}", bufs=2) as atpool,
            tc.tile_pool(name=f"e{rep}", bufs=4) as epool,
            tc.tile_pool(name=f"go{rep}", bufs=1) as gopool,
            tc.tile_pool(name=f"small{rep}", bufs=2) as small,
            tc.tile_pool(name=f"psg{rep}", bufs=2, space="PSUM") as psg,
            tc.tile_pool(name=f"pss{rep}", bufs=2, space="PSUM") as pss,
            tc.tile_pool(name=f"psv{rep}", bufs=2, space="PSUM") as psv,
        ):
            # DRAM staging for the per-step AllGather exchange
            agin = [
                dram.tile([P, 2 * QB], BF16, tag="agin", name=f"agin{rep}_{t}")
                for t in range(3)
            ]
            agout = dram.tile([3 * TP * P, 2 * QB], BF16, tag="agout",
                              name=f"agout{rep}")

            # ---------- weight / bias / mask loads (Pool-queue SWDGE, so the
            # SP HWDGE path is free for the x streams) ----------
            wq_sb = wpool.tile([P, nf * DH], BF16, tag="wq")
            wk_sb = wpool.tile([P, nf * DH], BF16, tag="wk")
            wv_sb = wpool.tile([P, nf * DH], BF16, tag="wv")
            wvb_sb = wpool.tile([1, DH], BF16, tag="wvb")
            qkb_sb = wpool.tile([P, 4], F32, tag="qkb")
            beff_sb = wpool.tile([P, nf], F32, tag="beff")
            mask_sb = wpool.tile([P, SP], F32, tag="mask")
            mask_row = wpool.tile([1, SP], F32, tag="maskrow")
            gidx_sb = wpool.tile([P, 3 * 8], I16, tag="gidx")
            ones_sb = wpool.tile([1, 64], F32R, tag="ones")
            cpw_sb = wpool.tile([P, nf * H], BF16, tag="cpw")
            mw1_sb = wpool.tile([P, nf * H], BF16, tag="mw1")
            mw2_sb = wpool.tile([P, nf * H], BF16, tag="mw2")
            xmlp_sb = wpool.tile([P, nf * QB], BF16, tag="xmlp")
            xatd_sb = wpool.tile([P, nf * SP], BF16, tag="xatd")
            xaug_sb = wpool.tile([1, SP], BF16, tag="xaug")

            nc.gpsimd.dma_start(
                out=wk_sb[:].rearrange("p (t d) -> p t d", d=DH),
                in_=wk_d[:H].rearrange("(t p) d -> p t d", p=P),
            )
            nc.gpsimd.dma_start(out=qkb_sb[:], in_=qkb_d[:])
            nc.gpsimd.dma_start(out=mask_row[:], in_=kmask_d[0:1])
            nc.gpsimd.partition_broadcast(mask_sb[:], mask_row[:], channels=P)
            nc.gpsimd.dma_start(out=gidx_sb[:], in_=gidx_d[:])
            nc.gpsimd.dma_start(out=ones_sb[:], in_=ones_d[:])

            def emit_xatd_dmas():
                # attendee x (compacted), KB-column chunks so K(kb0) can
                # start after ~1/NKB of the transfer; queued on sync AFTER
                # the first xq pair (which gates QT(0)).
                for kb in range(NKB):
                    cs = slice(kb * KB, (kb + 1) * KB)
                    for i in range(2):
                        nc.sync.dma_start(
                            out=xatd_sb[:, i * (nf // 2) * SP : (i + 1) * (nf // 2) * SP]
                            .rearrange("p (t q) -> p t q", q=SP)[:, :, cs],
                            in_=xatd[i * (H // 2) : (i + 1) * (H // 2)]
                            .rearrange("(t p) q -> p t q", p=P)[:, :, cs],
                        )
                nc.gpsimd.dma_start(out=xaug_sb[:], in_=xatd[H : H + 1])
            nc.gpsimd.dma_start(
                out=wv_sb[:].rearrange("p (t d) -> p t d", d=DH),
                in_=wv_d[:H].rearrange("(t p) d -> p t d", p=P),
            )
            nc.gpsimd.dma_start(out=wvb_sb[:], in_=wv_d[H : H + 1])
            nc.gpsimd.dma_start(
                out=wq_sb[:].rearrange("p (t d) -> p t d", d=DH),
                in_=wq_d[:H].rearrange("(t p) d -> p t d", p=P),
            )

            def late_weight_dmas():
                nc.gpsimd.dma_start(
                    out=cpw_sb[:].rearrange("p (d o) -> p d o", o=H),
                    in_=cpw_d[:].rearrange("(d p) o -> p d o", p=P),
                )
                nc.gpsimd.dma_start(
                    out=xmlp_sb[:].rearrange("p (t q) -> p t q", q=QB),
                    in_=xmlp_d[:].rearrange("(t p) q -> p t q", p=P),
                )
                for w_d, w_sb in ((mw1_d, mw1_sb), (mw2_d, mw2_sb)):
                    nc.gpsimd.dma_start(
                        out=w_sb[:].rearrange("p (t o) -> p t o", o=H),
                        in_=w_d[:].rearrange("(t p) o -> p t o", p=P),
                    )
                nc.gpsimd.dma_start(out=beff_sb[:], in_=beff_d[:])

            # ---------- persistent activations ----------
            KT_sb = per.tile([P, 2 * SP], BF16, tag="kt")    # pair p at cols p*SP
            V_sb = per.tile([P, NKT * 260], BF16, tag="v")   # per kt: 4 heads x 65
            out1_sb = per.tile([P, nf * QB], BF16, tag="out1")  # ot*QB + q
            attrcv = per.tile([P, 3 * 2 * QB], BF16, tag="attrcv")
            zpart_sb = per.tile([P, nf * QB], BF16, tag="zpart")
            z_sb = per.tile([P, nf * QB], BF16, tag="zs")

            # denominator column of augmented V (1 / N_masked / 0 per slot)
            for h in range(HPC):
                nc.gpsimd.dma_start(
                    out=V_sb[:].rearrange("p (t x) -> p t x", x=260)
                    [:, :, h * 65 + 64 : h * 65 + 65],
                    in_=vcol_d[:].rearrange("p (t o) -> p t o", o=1),
                )

            # ---------- emission units ----------
            def kv_k_unit(kb, p):
                def emit():
                    cs = slice(kb * KB, (kb + 1) * KB)
                    ps = psg.tile([P, QB], F32, tag="g", name=f"kps{kb}_{p}")
                    for t in range(nf):
                        nc.tensor.matmul(
                            ps[:, :KB],
                            lhsT=w_slice(wk_sb, t, p),
                            rhs=xatd_sb[:, t * SP + kb * KB : t * SP + (kb + 1) * KB],
                            start=(t == 0),
                            stop=(t == nf - 1),
                        )
                    # evacuation: (k + bias) * mask in one DVE op
                    nc.vector.scalar_tensor_tensor(
                        KT_sb[:, p * SP + kb * KB : p * SP + (kb + 1) * KB],
                        ps[:, :KB],
                        qkb_sb[:, 2 + p : 3 + p],
                        mask_sb[:, cs],
                        ALU.add,
                        ALU.mult,
                    )
                return emit

            def kv_v_unit(kt):
                def emit():
                    pv_ps = psg.tile([P, QB], F32, tag="g", name=f"vps{kt}")
                    for t in range(nf):
                        nc.tensor.matmul(
                            pv_ps[:, :DH],
                            lhsT=xatd_sb[:, t * SP + kt * P : t * SP + (kt + 1) * P],
                            rhs=wv_sb[:, t * DH : (t + 1) * DH],
                            start=(t == 0),
                            stop=False,
                        )
                    nc.tensor.matmul(
                        pv_ps[:, :DH],
                        lhsT=xaug_sb[0:1, kt * P : (kt + 1) * P],
                        rhs=wvb_sb[:],
                        start=False,
                        stop=True,
                    )
                    nc.vector.tensor_copy(
                        V_sb[:, kt * 260 : (kt + 1) * 260]
                        .rearrange("p (h c) -> p h c", c=65)[:, :, 0:64],
                        pv_ps[:, :DH].rearrange("p (h c) -> p h c", c=HD),
                    )
                return emit

            xq_chs = {}

            def qt_units(qb):
                """QT output-half units for qb; even qb's first unit DMAs the
                1024-col x pair (qb, qb+1)."""
                qo = qb % 2
                if qb % 2 == 0:
                    xq_chs[qb // 2] = [
                        xstream.tile([P, (nf // 2) * 2 * QB], BF16, tag="xch",
                                     name=f"xq{qb}_{i}")
                        for i in range(2)
                    ]
                x_ch = xq_chs[qb // 2]
                QT_t = qtpool.tile([P, 2 * QB], BF16, tag="qt",
                                   name=f"qt{qb}")
                def emit_p(p, dma=False):
                    def emit():
                        if dma:
                            cs = slice((qb // 2) * 2 * QB,
                                       (qb // 2 + 1) * 2 * QB)
                            for i in range(2):
                                nc.sync.dma_start(
                                    out=x_ch[i][:].rearrange(
                                        "p (t q) -> p t q", q=2 * QB),
                                    in_=xatt[i * (H // 2) : (i + 1) * (H // 2)]
                                    .rearrange("(t p) q -> p t q", p=P)[:, :, cs],
                                )
                        ps = psg.tile([P, QB], F32, tag="g", name=f"qps{qb}_{p}")
                        for t in range(nf):
                            nc.tensor.matmul(
                                ps[:],
                                lhsT=w_slice(wq_sb, t, p),
                                rhs=x_ch[t // 4][
                                    :, (t % 4) * 2 * QB + qo * QB :
                                    (t % 4) * 2 * QB + (qo + 1) * QB],
                                start=(t == 0),
                                stop=(t == nf - 1),
                            )
                        nc.vector.tensor_scalar(
                            QT_t[:, p * QB : (p + 1) * QB],
                            ps[:],
                            qkb_sb[:, p : p + 1],
                            None,
                            ALU.add,
                        )
                    return emit
                return QT_t, [emit_p(0, dma=(qb % 2 == 0)), emit_p(1)]

            def mlp1_unit(ot):
                def emit():
                    ps = psg.tile([P, QB], F32, tag="g", name=f"m1ps{ot}")
                    for t in range(nf):
                        nc.tensor.matmul(
                            ps[:],
                            lhsT=mw1_sb[:, t * H + ot * P : t * H + (ot + 1) * P],
                            rhs=xmlp_sb[:, t * QB : (t + 1) * QB],
                            start=(t == 0),
                            stop=(t == nf - 1),
                        )
                    nc.vector.tensor_copy(
                        out1_sb[:, ot * QB : (ot + 1) * QB], ps[:]
                    )
                return emit

            fillers = []

            def pop_filler():
                if fillers:
                    fillers.pop(0)()

            def attention(tq, QT_t, attnT_t, pre_kt=None):
                for p in range(2):
                    pvs = [
                        psv.tile([65, QB], F32, tag="pv",
                                 name=f"pv{tq}_{p}_{h}")
                        for h in range(2)
                    ]
                    sc_tiles = {}

                    def emit_scores(kt, p=p, tq=tq, QT_t=QT_t,
                                    sc_tiles=sc_tiles):
                        if pre_kt is not None:
                            pre_kt(kt)
                        sc = pss.tile([P, 2 * QB], F32, tag="sc",
                                      name=f"sc{tq}_{p}_{kt}")
                        sc_tiles[kt] = sc
                        for half in range(2):
                            nc.tensor.matmul(
                                sc[:, half * QB : (half + 1) * QB],
                                lhsT=KT_sb[
                                    64 * half : 64 * half + 64,
                                    p * SP + kt * P : p * SP + (kt + 1) * P,
                                ],
                                rhs=QT_t[64 * half : 64 * half + 64,
                                         p * QB : (p + 1) * QB],
                                start=True,
                                stop=True,
                                tile_position=(64 * half, 0),
                            )

                    emit_scores(0)
                    for kt in range(NKT):
                        if kt + 1 < NKT:
                            emit_scores(kt + 1)
                        pop_filler()
                        e = epool.tile([P, 2 * QB], BF16, tag="e")
                        nc.scalar.activation(e[:], sc_tiles.pop(kt)[:], AF.Exp)
                        for half in range(2):
                            h = 2 * p + half
                            nc.tensor.matmul(
                                pvs[half][:],
                                lhsT=V_sb[:, kt * 260 + h * 65 : kt * 260
                                          + (h + 1) * 65],
                                rhs=e[:, half * QB : (half + 1) * QB],
                                start=(kt == 0),
                                stop=(kt == NKT - 1),
                            )
                    # normalize by the denominator (row 64), store attn^T.
                    # rec broadcast via a PE ones-matmul (NOT gpsimd, which
                    # hosts the collectives and may block).
                    for half in range(2):
                        rec = small.tile([1, QB], F32R, tag="rec")
                        with nc.allow_low_precision(
                            reason="fp32r reciprocal for PE broadcast; "
                            "2^-11 rel err on softmax denom is fine"
                        ):
                            nc.vector.reciprocal(rec[:], pvs[half][64:65, :])
                        recb = psg.tile([P, QB], F32, tag="g",
                                        name=f"recb{tq}_{p}_{half}")
                        nc.tensor.matmul(
                            recb[0:64, :],
                            lhsT=ones_sb[:],
                            rhs=rec[:],
                            start=True,
                            stop=True,
                        )
                        recs = small.tile([64, QB], F32, tag="recs")
                        nc.vector.tensor_copy(recs[:], recb[0:64, :])
                        nc.vector.tensor_tensor(
                            attnT_t[64 * half : 64 * half + 64,
                                    p * QB : (p + 1) * QB],
                            pvs[half][0:64, :],
                            recs[:],
                            ALU.mult,
                        )

            # ---------- emission schedule ----------
            QT_ts = {}
            QT_ts[0], qt0_units = qt_units(0)
            qt0_units[0]()          # emits the xq pair-0 DMA first
            emit_xatd_dmas()
            qt0_units[1]()

            kunits = {kb: [kv_k_unit(kb, 0), kv_k_unit(kb, 1)]
                      for kb in range(NKB)}
            vunits = {kt: kv_v_unit(kt) for kt in range(NKT)}

            def ensure_kv(kt):
                """Emit K-block and V units needed by scores/attnV at kt."""
                for b in range(min(kt // 3 + 1, NKB)):
                    for u in kunits.pop(b, ()):
                        u()
                for k2 in range(kt + 1):
                    u = vunits.pop(k2, None)
                    if u is not None:
                        u()

            def cproj_rhs(d):
                sl, pp = d // 2, d % 2
                return (
                    attrcv[:, sl * 2 * QB + pp * QB : sl * 2 * QB + (pp + 1) * QB]
                    if sl < 3
                    else attnT_ts[3][:, pp * QB : (pp + 1) * QB]
                )

            def cproj_a_unit(ot):
                """Contraction slots 0-1 (pieces gathered after steps 0/1);
                runs as PE filler during attention step 3."""
                def emit():
                    ps = psg.tile([P, QB], F32, tag="g", name=f"cpa{ot}")
                    for d in range(4):
                        nc.tensor.matmul(
                            ps[:],
                            lhsT=cpw_sb[:, d * H + ot * P : d * H + (ot + 1) * P],
                            rhs=cproj_rhs(d),
                            start=(d == 0),
                            stop=(d == 3),
                        )
                    nc.vector.tensor_copy(
                        zpart_sb[:, ot * QB : (ot + 1) * QB], ps[:]
                    )
                return emit

            mlp1_per_qb = {1: [0, 1, 2], 2: [3, 4, 5], 3: [6, 7]}
            attnT_ts = {}
            for tq in range(nq):
                if tq == 1:
                    late_weight_dmas()
                if tq + 1 < nq:
                    QT_ts[tq + 1], units = qt_units(tq + 1)
                    fillers.extend(units)
                for ot in mlp1_per_qb.get(tq, []):
                    fillers.append(mlp1_unit(ot))
                if tq == 3:
                    fillers.extend(cproj_a_unit(ot) for ot in range(nf))
                attnT_ts[tq] = atpool.tile([P, 2 * QB], BF16, tag="at",
                                           name=f"at{tq}")
                attention(tq, QT_ts[tq], attnT_ts[tq],
                          pre_kt=ensure_kv if tq == 0 else None)
                if tq < 3:
                    # publish the piece + AllGather it (overlaps later steps),
                    # then pull this core's rank-row via dma_gather
                    nc.sync.dma_start(out=agin[tq][:], in_=attnT_ts[tq][:])
                    if skip_ag:
                        nc.gpsimd.dma_start(
                            out=attrcv[:, tq * 2 * QB : (tq + 1) * 2 * QB],
                            in_=agin[tq][:],
                        )
                    else:
                        nc.gpsimd.collective_compute(
                            "AllGather", ALU.bypass, replica_groups=rg,
                            ins=[agin[tq][:].opt()],
                            outs=[agout[tq * TP * P : (tq + 1) * TP * P, :].opt()],
                        )
                        nc.gpsimd.dma_gather(
                            attrcv[:, tq * 2 * QB : (tq + 1) * 2 * QB]
                            .rearrange("p (o c) -> p o c", o=1),
                            agout[tq * TP * P : (tq + 1) * TP * P, :],
                            gidx_sb[:, tq * 8 : (tq + 1) * 8],
                            P,
                            P,
                            2 * QB,
                        )
            while fillers:
                pop_filler()

            # ---------- c_proj tail: slots 2-3, add the filler partial ------
            for ot in range(nf):
                ps = psg.tile([P, QB], F32, tag="g", name=f"cpb{ot}")
                for d in range(4, nf):
                    nc.tensor.matmul(
                        ps[:],
                        lhsT=cpw_sb[:, d * H + ot * P : d * H + (ot + 1) * P],
                        rhs=cproj_rhs(d),
                        start=(d == 4),
                        stop=(d == nf - 1),
                    )
                nc.vector.tensor_tensor(
                    z_sb[:, ot * QB : (ot + 1) * QB],
                    ps[:],
                    zpart_sb[:, ot * QB : (ot + 1) * QB],
                    ALU.add,
                )

            # ---------- mlp2 (full 1024 outs, own q-quarter) ----------
            gout = gopool.tile([P, 2 * QB], BF16, tag="gout")
            for ot in range(nf):
                ps = psg.tile([P, QB], F32, tag="g", name=f"m2ps{ot}")
                for t in range(nf):
                    nc.tensor.matmul(
                        ps[:],
                        lhsT=mw2_sb[:, t * H + ot * P : t * H + (ot + 1) * P],
                        rhs=z_sb[:, t * QB : (t + 1) * QB],
                        start=(t == 0),
                        stop=(t == nf - 1),
                    )
                o1 = out1_sb[:, ot * QB : (ot + 1) * QB]
                nc.vector.tensor_add(o1, ps[:], o1)

                # gelu (+ folded mlp/cproj bias), batched output DMA
                g = gout[:, (ot % 2) * QB : (ot % 2 + 1) * QB]
                nc.scalar.activation(
                    g, o1, AF.Gelu_apprx_tanh, bias=beff_sb[:, ot : ot + 1]
                )
                if ot % 2 == 1:
                    nc.sync.dma_start(
                        out=outQ[(ot - 1) * P : (ot + 1) * P, :]
                        .rearrange("(t p) q -> p t q", p=P),
                        in_=gout[:].rearrange("p (t q) -> p t q", q=QB),
                    )

    nc.compile()
    return nc


def w_slice(w_sb, t, p):
    """lhsT [128, 128] slice: f-tile t, output half p, of a [128, nt*256] layout."""
    return w_sb[:, t * DH + p * P : t * DH + (p + 1) * P]


_NC_CACHE = {}
LAST_RESULTS = None


def _get_nc_reps(reps, sp=SP_OPTIONS[0]):
    key = ("reps", reps, sp)
    if key not in _NC_CACHE:
        _NC_CACHE[key] = _build_nc(reps=reps, sp=sp)
    return _NC_CACHE[key]


def _pick_sp(inputs):
    mask = np.asarray(inputs["attendee_mask"]).astype(bool)
    need = int(mask.sum(1).max()) + 1  # unmasked slots + phantom
    for sp in SP_OPTIONS:
        if need <= sp:
            return sp
    raise AssertionError(f"mask needs {need} slots > {SP_OPTIONS[-1]}")


def kernel(**inputs):
    global LAST_RESULTS
    sp = _pick_sp(inputs)
    nc = _get_nc_reps(1, sp)
    in_maps = make_in_maps(inputs, sp)

    trace = bool(int(os.environ.get("KERNEL_TRACE", "0")))
    res = bass_utils.run_bass_kernel_spmd(
        nc, in_maps, core_ids=list(range(NCORES)), trace=trace
    )
    LAST_RESULTS = res

    out = np.empty((B, S, H), np.float32)
    for c in range(NCORES):
        b, g = c // TP, c % TP
        out[b, g * QB : (g + 1) * QB, :] = (
            res.results[c]["outQ"].astype(np.float32).T)
    return out


def make_in_maps(inputs, sp=SP_OPTIONS[0]):
    SP = sp
    NKT = SP // P
    xq = np.ascontiguousarray(np.asarray(inputs["attender_seq"], np.float32))
    xk = np.ascontiguousarray(np.asarray(inputs["attendee_seq"], np.float32))
    mask = np.asarray(inputs["attendee_mask"]).astype(bool)
    caw = np.asarray(inputs["c_attn_w"], np.float32)
    cab = np.asarray(inputs["c_attn_b"], np.float32)
    cpw = np.ascontiguousarray(np.asarray(inputs["c_proj_w"], np.float32))
    cpb = np.asarray(inputs["c_proj_b"], np.float32)
    mw = np.ascontiguousarray(np.asarray(inputs["mlp_w"], np.float32))
    mb = np.asarray(inputs["mlp_b"], np.float32)

    mw1_bf = mw[:H].astype(NPBF16)
    mw2_bf = mw[H:].astype(NPBF16)
    # gelu bias: mlp_b + c_proj_b @ mlp_w2  (folded host-side)
    beff = (
        mb.astype(np.float64) + cpb.astype(np.float64) @ mw[H:].astype(np.float64)
    ).astype(np.float32)
    beff_t = np.ascontiguousarray(beff.reshape(H // P, P).T)
    cpw_bf = cpw.astype(NPBF16)

    # per-batch compaction of the attendee axis
    batch_atd = []
    for b in range(B):
        idx = np.flatnonzero(mask[b])
        n_u = len(idx)
        n_masked = S - n_u
        assert n_u + 1 <= SP, (n_u, SP)
        xatd = np.zeros((H + 1, SP), np.float32)
        xatd[:H, :n_u] = xk[b, idx].T
        xatd[:H, n_u] = xk[b].sum(0) - xk[b, idx].sum(0)  # sum of masked x
        xatd[H, :n_u] = 1.0
        xatd[H, n_u] = float(n_masked)
        kmask = np.zeros((1, SP), np.float32)
        kmask[0, :n_u] = 1.0
        vcol = np.zeros(SP, np.float32)
        vcol[:n_u] = 1.0
        vcol[n_u] = float(n_masked)
        vcol_t = np.ascontiguousarray(vcol.reshape(NKT, P).T.astype(NPBF16))
        batch_atd.append((xatd.astype(NPBF16), kmask, vcol_t))

    in_maps = []
    for c in range(NCORES):
        b, g = c // TP, c % TP
        gs = slice(g * DH, (g + 1) * DH)
        xatd_bf, kmask, vcol_t = batch_atd[b]
        wv = np.concatenate(
            [caw[:, 2 * H + g * DH : 2 * H + (g + 1) * DH],
             cab[None, 2 * H + g * DH : 2 * H + (g + 1) * DH]], 0)
        # [128, 4]: q bias (2 output halves), k bias (2 output halves)
        qkb = np.stack(
            [cab[gs][: P], cab[gs][P:],
             cab[H + g * DH : H + (g + 1) * DH][: P],
             cab[H + g * DH : H + (g + 1) * DH][P:]], 1)

        # virtual q-block order: own quarter LAST
        vorder = [(g + 1 + t) % TP for t in range(TP)]
        xatt_v = np.concatenate(
            [xq[b, vq * QB : (vq + 1) * QB, :] for vq in vorder], 0).T

        # receive plan: step t's gathered buffer -> rank r=(g-1-t)%4's piece;
        # c_proj contraction slot s uses cpw rows of that rank's dims
        # (slot 3 = own piece, local).
        rts = [(g - 1 - t) % TP for t in range(3)]
        cpw_v = np.concatenate(
            [cpw_bf[r * DH : (r + 1) * DH] for r in rts + [g]], 0)
        gidx = np.zeros((P, 24), np.int16)
        for t in range(3):
            vals = rts[t] * P + np.arange(P, dtype=np.int16)  # rows in buffer t
            for i in range(P):
                gidx[i % 16, t * 8 + i // 16] = vals[i]
        gidx[16:, :] = np.tile(gidx[:16, :], (7, 1))

        in_maps.append({
            "xatt": np.ascontiguousarray(xatt_v.astype(NPBF16)),
            "xatd": np.ascontiguousarray(xatd_bf),
            "xmlp": np.ascontiguousarray(
                xq[b, g * QB : (g + 1) * QB, :].T.astype(NPBF16)),
            "wq": np.ascontiguousarray(caw[:, gs].astype(NPBF16)),
            "wk": np.ascontiguousarray(
                caw[:, H + g * DH : H + (g + 1) * DH].astype(NPBF16)),
            "wv": np.ascontiguousarray(wv.astype(NPBF16)),
            "qkbias": np.ascontiguousarray(qkb),
            "beff": beff_t,
            "kmask": kmask,
            "vcol": vcol_t,
            "gidx": gidx,
            "ones1": np.ones((1, 64), np.float32),
            "cprojw": np.ascontiguousarray(cpw_v),
            "mlpw1": mw1_bf,
            "mlpw2": mw2_bf,
        })
    return in_maps
